# revision 2
# baseline (speedup 1.0000x reference)
"""Trainium2 Bass kernel for Gaussian-KDE logsumexp (nn_GaussianKernel).

out[n] = logsumexp_m( -0.5*||(y_n - x_m)/bw||^2 - Z ),  Z = D/2*log(2pi) + D*log(bw) + log(M)

On-device factorization (per query row n, data col m):
    A[n,m] = (y_n/bw^2) . x_m  +  c_m,       c_m = -||x_m||^2/(2 bw^2)   (host, fp64)
    out[n] = logsumexp_m A[n,m] + r_n,       r_n = -||y_n||^2/(2 bw^2) - Z (host, fp64)

y and x are quantized to bf16 once on the host; c_m / r_n are computed from the
quantized values so the on-device result is the exact logsumexp of slightly
perturbed points (error ~1e-3 relative, tolerance is 2e-2).

Sharding: data-parallel over the 2048 query rows -> 8 cores x 256 rows
(2 M-tiles of 128 partitions), each core holds the full x (K=D=128).

Per core: A is built per 512-col PSUM bank as a rank-1 f32r bias pass
(ones^T (x) crow, start=True) + a bf16 y.x pass (stop=True).  Work is split
into 4 chunks of 1024 cols ((mt, half) pairs) that pipeline PE -> DVE chunk
max -> ACT exp(+accum).  Chunk partials merge with an exact logsumexp merge.
ln() is a bitwise log2 approximation on DVE (err ~0.03 << tolerance), so the
only ACT table is exp, preloaded by a dummy activation at kernel start.
"""

import sys
from math import log, pi

import numpy as np
import ml_dtypes

sys.path.insert(0, "/opt/trn_rl_repo")

import concourse.bacc as bacc
import concourse.bass as bass
import concourse.mybir as mybir
import concourse.tile as tile
from concourse.bass_utils import run_bass_kernel_spmd

BW = 0.1
N_QUERY = 2048
N_DATA = 2048
DIM = 128
N_CORES = 8
SHARD = N_QUERY // N_CORES  # 256 query rows per core

Z_CONST = 0.5 * DIM * log(2.0 * pi) + DIM * log(BW) + log(float(N_DATA))

NB = 512                 # one PSUM bank of fp32
N_BANKS = N_DATA // NB   # 4
M_TILES = SHARD // 128   # 2
CHUNK = 1024             # pipeline granularity (2 banks)
N_CHUNKS = N_DATA // CHUNK  # 2 per M-tile

LN2 = 0.6931471805599453
# ln(S) ~= (int_bits(S) * 2^-23 - 127 + 0.0430357) * ln2
LOG_S1 = LN2 / (1 << 23)
LOG_S2 = (0.0430357 - 127.0) * LN2

_CACHE = {}


def _build_nc():
    f32 = mybir.dt.float32
    f32r = mybir.dt.float32r
    bf16 = mybir.dt.bfloat16
    i32 = mybir.dt.int32
    fx = mybir.ActivationFunctionType
    nc = bacc.Bacc("TRN2", target_bir_lowering=False, debug=False)

    xt = nc.dram_tensor("xt", [DIM, N_DATA], bf16, kind="ExternalInput")
    yt = nc.dram_tensor("yt", [DIM, SHARD], bf16, kind="ExternalInput")
    crow_d = nc.dram_tensor("crow", [1, N_DATA], f32r, kind="ExternalInput")
    rvec_d = nc.dram_tensor("rvec", [128, M_TILES], f32, kind="ExternalInput")
    out = nc.dram_tensor("out", [128, M_TILES], f32, kind="ExternalOutput")

    with tile.TileContext(nc) as tc:
        with (
            tc.tile_pool(name="io", bufs=1) as io,
            tc.tile_pool(name="psum", bufs=2, space=bass.MemorySpace.PSUM) as psum,
            tc.tile_pool(name="work", bufs=1) as work,
            tc.tile_pool(name="small", bufs=2) as small,
        ):
            # ---- dummy exp: pulls the ACT exp table load off the critical path
            dmy = small.tile([1, 1], f32, tag="dmy")
            dmy2 = small.tile([1, 1], f32, tag="dmy2")
            nc.gpsimd.memset(dmy[:], 0.0)
            nc.scalar.activation(dmy2[:], dmy[:], fx.Exp)

            # ---- ones row for the rank-1 bias pass
            ones = io.tile([1, 128], f32r, tag="ones")
            nc.gpsimd.memset(ones[:].bitcast(f32), 1.0)

            # ---- input DMAs (sync engine; crow first: bias passes need it)
            crow = io.tile([1, N_DATA], f32r, tag="crow")
            nc.sync.dma_start(crow[:], crow_d[:])
            yt_sb = io.tile([DIM, SHARD], bf16, tag="yt")
            nc.sync.dma_start(yt_sb[:], yt[:])
            xt_sb = io.tile([DIM, N_DATA], bf16, tag="xt")
            for h in range(N_CHUNKS):
                nc.sync.dma_start(xt_sb[:, h * CHUNK:(h + 1) * CHUNK],
                                  xt[:, h * CHUNK:(h + 1) * CHUNK])
            rvec = io.tile([128, M_TILES], f32, tag="rvec")
            nc.scalar.dma_start(rvec[:], rvec_d[:])

            esc = work.tile([128, CHUNK], bf16, tag="esc")

            A = [psum.tile([128, N_DATA], f32, tag="A", name=f"A{mt}")
                 for mt in range(M_TILES)]
            nmax = [[small.tile([128, 1], f32, tag="nmax", name=f"nmax{mt}_{h}")
                     for h in range(N_CHUNKS)] for mt in range(M_TILES)]
            tpack = [small.tile([128, N_CHUNKS], f32, tag="tpack", name=f"tp{mt}")
                     for mt in range(M_TILES)]
            osb = small.tile([128, M_TILES], f32, tag="osb")

            # ---- PE: per (mt, chunk): rank-1 bias pass + bf16 main pass
            for mt in range(M_TILES):
                for h in range(N_CHUNKS):
                    for b in range(h * CHUNK // NB, (h + 1) * CHUNK // NB):
                        nc.tensor.matmul(A[mt][:, b * NB:(b + 1) * NB],
                                         ones[:],
                                         crow[:, b * NB:(b + 1) * NB],
                                         start=True, stop=False)
                    for b in range(h * CHUNK // NB, (h + 1) * CHUNK // NB):
                        nc.tensor.matmul(A[mt][:, b * NB:(b + 1) * NB],
                                         yt_sb[:, mt * 128:(mt + 1) * 128],
                                         xt_sb[:, b * NB:(b + 1) * NB],
                                         start=False, stop=True)

                    # ---- DVE: chunk -max;  ACT: exp(A - max) with row-sum
                    Ach = A[mt][:, h * CHUNK:(h + 1) * CHUNK]
                    nc.vector.tensor_reduce(nmax[mt][h][:], Ach,
                                            axis=mybir.AxisListType.X,
                                            op=mybir.AluOpType.max, negate=True)
                    nc.scalar.activation(esc[:], Ach, fx.Exp,
                                         bias=nmax[mt][h][:], scale=1.0,
                                         accum_out=tpack[mt][:, h:h + 1])

            # ---- merge chunk partials per M-tile and store
            for mt in range(M_TILES):
                nmin = small.tile([128, 1], f32, tag="nmin", name=f"nmin{mt}")
                nc.vector.tensor_tensor(nmin[:], nmax[mt][0][:], nmax[mt][1][:],
                                        op=mybir.AluOpType.min)
                dpack = small.tile([128, N_CHUNKS], f32, tag="dpack",
                                   name=f"dp{mt}")
                for h in range(N_CHUNKS):
                    nc.vector.tensor_sub(dpack[:, h:h + 1], nmin[:],
                                         nmax[mt][h][:])
                wpack = small.tile([128, N_CHUNKS], f32, tag="wpack",
                                   name=f"wp{mt}")
                nc.scalar.activation(wpack[:], dpack[:], fx.Exp)
                sw = small.tile([128, N_CHUNKS], f32, tag="sw", name=f"sw{mt}")
                nc.vector.tensor_tensor(sw[:], tpack[mt][:], wpack[:],
                                        op=mybir.AluOpType.mult)
                S = small.tile([128, 1], f32, tag="S", name=f"S{mt}")
                nc.vector.tensor_reduce(S[:], sw[:],
                                        axis=mybir.AxisListType.X,
                                        op=mybir.AluOpType.add)
                # ln(S) via bitwise log2 (error ~0.03 absolute)
                sbits = small.tile([128, 1], f32, tag="sbits", name=f"sb{mt}")
                nc.vector.tensor_copy(sbits[:], S[:].bitcast(i32))
                lns = small.tile([128, 1], f32, tag="lns", name=f"ln{mt}")
                nc.vector.tensor_scalar(lns[:], sbits[:], LOG_S1, LOG_S2,
                                        op0=mybir.AluOpType.mult,
                                        op1=mybir.AluOpType.add)
                t1 = small.tile([128, 1], f32, tag="t1", name=f"t1_{mt}")
                nc.vector.tensor_sub(t1[:], lns[:], nmin[:])
                nc.vector.tensor_add(osb[:, mt:mt + 1], t1[:],
                                     rvec[:, mt:mt + 1])
                nc.sync.dma_start(out[:, mt:mt + 1], osb[:, mt:mt + 1])

    nc.compile()
    return nc


def make_in_maps(y, x):
    """Host-side prep: bf16 quantization + fp64 norm corrections."""
    y = np.asarray(y, dtype=np.float32)
    x = np.asarray(x, dtype=np.float32)

    xq = x.astype(ml_dtypes.bfloat16)                       # (M, D) bf16
    xt = np.ascontiguousarray(xq.T)                         # (D, M) bf16
    # c_m from the quantized x actually used on device
    xq64 = xq.astype(np.float64)
    crow = (-0.5 / (BW * BW)) * np.sum(xq64 * xq64, axis=1)  # (M,)
    crow = crow.reshape(1, N_DATA).astype(np.float32)

    in_maps = []
    for i in range(N_CORES):
        ysh = y[i * SHARD:(i + 1) * SHARD]
        ytq = (ysh.astype(np.float64) / (BW * BW)).astype(ml_dtypes.bfloat16)
        # effective y-hat = ytq * bw^2;  r_n = -||y-hat||^2/(2 bw^2) - Z
        yt64 = ytq.astype(np.float64)
        r = -0.5 * (BW * BW) * np.sum(yt64 * yt64, axis=1) - Z_CONST  # (SHARD,)
        rvec = np.ascontiguousarray(
            r.reshape(M_TILES, 128).T).astype(np.float32)   # (128, M_TILES)
        in_maps.append({
            "xt": xt,
            "yt": np.ascontiguousarray(ytq.T),              # (D, SHARD) bf16
            "crow": crow,
            "rvec": rvec,
        })
    return in_maps


def kernel(y, x):
    assert np.asarray(y).shape == (N_QUERY, DIM)
    assert np.asarray(x).shape == (N_DATA, DIM)

    if "nc" not in _CACHE:
        _CACHE["nc"] = _build_nc()
    nc = _CACHE["nc"]

    in_maps = make_in_maps(y, x)
    res = run_bass_kernel_spmd(nc, in_maps, core_ids=list(range(N_CORES)))
    # out[p, mt] holds query row mt*128+p of the core's shard
    return np.concatenate(
        [r["out"].T.reshape(-1) for r in res.results]).astype(np.float32)


# revision 4
# speedup vs baseline: 1.3770x; 1.3770x over previous
"""Trainium2 Bass kernel for Gaussian-KDE logsumexp (nn_GaussianKernel).

out[n] = logsumexp_m( -0.5*||(y_n - x_m)/bw||^2 - Z ),  Z = D/2*log(2pi) + D*log(bw) + log(M)

On-device factorization (per query row n, data col m):
    A[n,m] = (y_n/bw^2) . x_m  +  c_m,       c_m = -||x_m||^2/(2 bw^2)   (host, fp64)
    out[n] = logsumexp_m A[n,m] + r_n,       r_n = -||y_n||^2/(2 bw^2) - Z (host, fp64)

y and x are quantized to bf16 once on the host; c_m / r_n are computed from the
quantized values, so the device result is the exact logsumexp of slightly
perturbed points (error ~1e-3 relative; tolerance 2e-2).  c_m rides into PSUM
as a K=2 rank-2 bf16 matmul (ones^T @ [c_hi; c_lo], hi/lo bf16 split keeps c
accurate to ~0.03).

Sharding: data-parallel over the 2048 query rows -> 8 cores x 256 rows
(2 M-tiles of 128 partitions), each core holds the full x (K=D=128).

Per core: warmup matmuls run during the input-DMA wait to lift the PE HAM
clock gate; per M-tile the bias pass (start=True) + bf16 y.x pass (stop=True)
fill a [128,2048] PSUM tile, then DVE does one negated row-max, ACT does one
exp with fused row-sum accumulation, and ln() is a bitwise log2 approximation
on DVE (err ~0.03), so the only ACT table is exp, preloaded by a dummy
activation at kernel start.
"""

import sys
from math import log, pi

import numpy as np
import ml_dtypes

sys.path.insert(0, "/opt/trn_rl_repo")

import concourse.bacc as bacc
import concourse.bass as bass
import concourse.mybir as mybir
import concourse.tile as tile
from concourse.bass_utils import run_bass_kernel_spmd

BW = 0.1
N_QUERY = 2048
N_DATA = 2048
DIM = 128
N_CORES = 8
SHARD = N_QUERY // N_CORES  # 256 query rows per core

Z_CONST = 0.5 * DIM * log(2.0 * pi) + DIM * log(BW) + log(float(N_DATA))

NM = 512                   # matmul free-dim (one fp32 PSUM bank)
M_TILES = SHARD // 128     # 2
N_WARMUP = 4               # PE warmup matmuls (N=512 each) during DMA wait

LN2 = 0.6931471805599453
# ln(S) ~= (int_bits(S) * 2^-23 - 127 + 0.0430357) * ln2
LOG_S1 = LN2 / (1 << 23)
LOG_S2 = (0.0430357 - 127.0) * LN2

_CACHE = {}


def _build_nc():
    f32 = mybir.dt.float32
    bf16 = mybir.dt.bfloat16
    i32 = mybir.dt.int32
    fx = mybir.ActivationFunctionType
    nc = bacc.Bacc("TRN2", target_bir_lowering=False, debug=False)

    xt = nc.dram_tensor("xt", [DIM, N_DATA], bf16, kind="ExternalInput")
    yt = nc.dram_tensor("yt", [DIM, SHARD], bf16, kind="ExternalInput")
    crow_d = nc.dram_tensor("crow", [2, N_DATA], bf16, kind="ExternalInput")
    rvec_d = nc.dram_tensor("rvec", [128, M_TILES], f32, kind="ExternalInput")
    out = nc.dram_tensor("out", [128, M_TILES], f32, kind="ExternalOutput")

    with tile.TileContext(nc) as tc:
        with (
            tc.tile_pool(name="io", bufs=1) as io,
            tc.tile_pool(name="psum", bufs=2, space=bass.MemorySpace.PSUM) as psum,
            tc.tile_pool(name="work", bufs=1) as work,
            tc.tile_pool(name="small", bufs=2) as small,
        ):
            # ---- dummy exp: pulls the ACT exp-table load off the critical path
            dmy = small.tile([1, 1], f32, tag="dmy")
            dmy2 = small.tile([1, 1], f32, tag="dmy2")
            nc.gpsimd.memset(dmy[:], 0.0)
            nc.scalar.activation(dmy2[:], dmy[:], fx.Exp)

            # ---- constants for the rank-2 bias pass + PE warmup fodder
            ones2 = io.tile([2, 128], bf16, tag="ones2")
            nc.gpsimd.memset(ones2[:], 1.0)
            junk2 = io.tile([2, 512], bf16, tag="junk2")
            nc.gpsimd.memset(junk2[:], 0.0)

            # ---- input DMAs (crow first: bias passes need only it)
            crow = io.tile([2, N_DATA], bf16, tag="crow")
            nc.sync.dma_start(crow[:], crow_d[:])
            xt_sb = io.tile([DIM, N_DATA], bf16, tag="xt")
            for h in range(N_DATA // NM):
                nc.sync.dma_start(xt_sb[:, h * NM:(h + 1) * NM],
                                  xt[:, h * NM:(h + 1) * NM])
            yt_sb = io.tile([DIM, SHARD], bf16, tag="yt")
            nc.scalar.dma_start(yt_sb[:], yt[:])
            rvec = io.tile([128, M_TILES], f32, tag="rvec")
            nc.scalar.dma_start(rvec[:], rvec_d[:])

            A = [psum.tile([128, N_DATA], f32, tag="A", name=f"A{mt}")
                 for mt in range(M_TILES)]

            # ---- PE warmup: garbage matmuls (overwritten by the bias pass)
            # keep the HAM activity window busy while input DMAs complete
            for w in range(N_WARMUP):
                nc.tensor.matmul(A[0][:, :512], ones2[:], junk2[:],
                                 start=True, stop=True)

            # ---- PE: per M-tile: rank-2 bias pass then bf16 main pass
            for mt in range(M_TILES):
                for h in range(N_DATA // NM):
                    nc.tensor.matmul(A[mt][:, h * NM:(h + 1) * NM],
                                     ones2[:],
                                     crow[:, h * NM:(h + 1) * NM],
                                     start=True, stop=False)
            for mt in range(M_TILES):
                for h in range(N_DATA // NM):
                    nc.tensor.matmul(A[mt][:, h * NM:(h + 1) * NM],
                                     yt_sb[:, mt * 128:(mt + 1) * 128],
                                     xt_sb[:, h * NM:(h + 1) * NM],
                                     start=False, stop=True)

            # ---- per M-tile: DVE row-max -> ACT exp(+accum) -> bit-log tail
            esc = work.tile([128, N_DATA], bf16, tag="esc")
            spack = small.tile([128, M_TILES], f32, tag="spack")
            osb = small.tile([128, M_TILES], f32, tag="osb")
            for mt in range(M_TILES):
                nmax = small.tile([128, 1], f32, tag="nmax", name=f"nmax{mt}")
                nc.vector.tensor_reduce(nmax[:], A[mt][:],
                                        axis=mybir.AxisListType.X,
                                        op=mybir.AluOpType.max, negate=True)
                radj = small.tile([128, 1], f32, tag="radj", name=f"radj{mt}")
                nc.vector.tensor_sub(radj[:], rvec[:, mt:mt + 1], nmax[:])
                nc.scalar.activation(esc[:], A[mt][:], fx.Exp,
                                     bias=nmax[:], scale=1.0,
                                     accum_out=spack[:, mt:mt + 1])
                # out = ln(S) - nmax + rvec';  ln(S) via bitwise log2
                sbits = small.tile([128, 1], f32, tag="sbits", name=f"sb{mt}")
                nc.vector.tensor_copy(sbits[:], spack[:, mt:mt + 1].bitcast(i32))
                nc.vector.tensor_scalar(osb[:, mt:mt + 1], sbits[:],
                                        LOG_S1, radj[:],
                                        op0=mybir.AluOpType.mult,
                                        op1=mybir.AluOpType.add)
                nc.sync.dma_start(out[:, mt:mt + 1], osb[:, mt:mt + 1])

    nc.compile()
    return nc


def make_in_maps(y, x):
    """Host-side prep: bf16 quantization + fp64 norm corrections."""
    y = np.asarray(y, dtype=np.float32)
    x = np.asarray(x, dtype=np.float32)

    xq = x.astype(ml_dtypes.bfloat16)                       # (M, D) bf16
    xt = np.ascontiguousarray(xq.T)                         # (D, M) bf16
    # c_m from the quantized x actually used on device; hi/lo bf16 split
    xq64 = xq.astype(np.float64)
    c = (-0.5 / (BW * BW)) * np.sum(xq64 * xq64, axis=1)    # (M,) fp64
    c_hi = c.astype(ml_dtypes.bfloat16)
    c_lo = (c - c_hi.astype(np.float64)).astype(ml_dtypes.bfloat16)
    crow = np.ascontiguousarray(np.stack([c_hi, c_lo]))     # (2, M) bf16

    in_maps = []
    for i in range(N_CORES):
        ysh = y[i * SHARD:(i + 1) * SHARD]
        ytq = (ysh.astype(np.float64) / (BW * BW)).astype(ml_dtypes.bfloat16)
        # effective y-hat = ytq * bw^2;  r_n = -||y-hat||^2/(2 bw^2) - Z
        yt64 = ytq.astype(np.float64)
        r = -0.5 * (BW * BW) * np.sum(yt64 * yt64, axis=1) - Z_CONST + LOG_S2
        rvec = np.ascontiguousarray(
            r.reshape(M_TILES, 128).T).astype(np.float32)   # (128, M_TILES)
        in_maps.append({
            "xt": xt,
            "yt": np.ascontiguousarray(ytq.T),              # (D, SHARD) bf16
            "crow": crow,
            "rvec": rvec,
        })
    return in_maps


def kernel(y, x):
    assert np.asarray(y).shape == (N_QUERY, DIM)
    assert np.asarray(x).shape == (N_DATA, DIM)

    if "nc" not in _CACHE:
        _CACHE["nc"] = _build_nc()
    nc = _CACHE["nc"]

    in_maps = make_in_maps(y, x)
    res = run_bass_kernel_spmd(nc, in_maps, core_ids=list(range(N_CORES)))
    # out[p, mt] holds query row mt*128+p of the core's shard
    return np.concatenate(
        [r["out"].T.reshape(-1) for r in res.results]).astype(np.float32)
